# revision 17
# baseline (speedup 1.0000x reference)
"""DeltaNet layer kernel for 8 Trainium2 NeuronCores.

Math note: in the reference's _delta_scan, the update added to the (D,D)
state h is identical for every row and h0=0, so all rows of h stay equal
forever. The layer therefore reduces exactly to a per-(batch, head)
first-order scalar-decay recurrence on a D-vector:

    c_t = beta_t * c_{t-1} + k_t * vsum_t,   o_t = qsum_t * c_t

with vsum = sum_d v, qsum = sum_d q. qsum/vsum only need x @ col-sums of
Wq/Wv. The recurrence maps 1:1 onto the DVE tensor_tensor_scan
instruction (fp32 state, one lane per (head, d) pair, scan along
tokens).

All GEMMs run in bf16 (fp32r measured ~2 cycles/row on HW vs bf16's 1;
bf16 also halves HBM traffic). Precision-sensitive pieces stay fp32:
the sigmoid/beta path (its error compounds through the recurrence, so
only pre-sigmoid LOGITS ever touch bf16), the scan state, and the k
tiles. One-shot values (qsum/vsum/o/residual/output) tolerate bf16.

Two SPMD launches on cores 0-7:
  L1: core (b, head-group of 8): GEMM1 -> extras (logit/qsum/vsum rows)
      + k rows; selector-matmul replication of the 3 extras fields to 64
      lanes/head (sigmoid fused into the beta evacuation, bias applied
      per-lane); u = k*vs; tensor_tensor_scan; o = qs*c -> bf16.
  L2: core (b, token-half): GEMM2 o @ Wo (b_o pre-folded into the
      residual host-side), fused residual-add + mean-accum on DVE,
      sumsq on ACT, LayerNorm, bf16 output upcast on host.
"""
import sys

sys.path.insert(0, "/opt/trn_rl_repo")

import numpy as np
import ml_dtypes

BF = ml_dtypes.bfloat16

B, S, HID, NH = 4, 4096, 1024, 16
D = HID // NH
EPS = 1e-5
HG = 8          # heads per L1 core
TH = S // 2     # tokens per L2 core


def _build_l1():
    import concourse.mybir as mybir
    from concourse import tile, bacc

    f32, bf16 = mybir.dt.float32, mybir.dt.bfloat16
    AF = mybir.ActivationFunctionType
    ALU = mybir.AluOpType

    nc = bacc.Bacc("TRN2", target_bir_lowering=False, debug=False, num_devices=8)
    xT = nc.dram_tensor("xT", [HID, S], bf16, kind="ExternalInput")
    # Wcat columns: [zb(8) | qs(8) | vs(8) | k(512)]
    Wcat = nc.dram_tensor("Wcat", [HID, 536], bf16, kind="ExternalInput")
    bbx = nc.dram_tensor("bbx", [128, 4], f32, kind="ExternalInput")
    # 12 selector blocks, order (mi, field): exp[p,:] row0+2mi+p//64 one-hot
    selc = nc.dram_tensor("selc", [24, 12 * 128], bf16, kind="ExternalInput")
    o_out = nc.dram_tensor("o_out", [HG * D, S], bf16, kind="ExternalOutput")

    KT = 8          # hid k-tiles
    NW = 512        # tokens per slab
    NS = S // NW

    with tile.TileContext(nc) as tc:
        with tc.tile_pool(name="wc", bufs=1) as wc_pool, \
             tc.tile_pool(name="xt", bufs=2) as xt_pool, \
             tc.tile_pool(name="ext", bufs=2) as ext_pool, \
             tc.tile_pool(name="ksb", bufs=2) as ksb_pool, \
             tc.tile_pool(name="bx", bufs=3) as bx_pool, \
             tc.tile_pool(name="qx", bufs=3) as qx_pool, \
             tc.tile_pool(name="u", bufs=3) as u_pool, \
             tc.tile_pool(name="c", bufs=2) as c_pool, \
             tc.tile_pool(name="osb", bufs=2) as o_pool, \
             tc.tile_pool(name="ps", bufs=4, space="PSUM") as ps_pool, \
             tc.tile_pool(name="psel", bufs=2, space="PSUM") as psel_pool:

            # weights on the ACT HWDGE queue so they overlap the first x
            # slab load on the sync queue; per-k tiles so the first matmul
            # only waits for chunk 0 (tile deps are whole-tile)
            wck = []
            for k in range(KT):
                w1 = wc_pool.tile([128, 536], bf16, tag=f"wc{k}", name=f"wc{k}")
                nc.scalar.dma_start(out=w1[:], in_=Wcat[k * 128:(k + 1) * 128, :])
                wck.append(w1)
            bb = wc_pool.tile([128, 4], f32)
            nc.gpsimd.dma_start(out=bb[:], in_=bbx[:])
            sel = wc_pool.tile([24, 12, 128], bf16)
            nc.gpsimd.dma_start(out=sel[:], in_=selc.rearrange("p (b l) -> p b l", l=128))

            xr_ = xT.rearrange("(kt p) s -> p kt s", p=128)

            def load_xt(s):
                t0 = s * NW
                lo = xt_pool.tile([128, 4, NW], bf16, tag="xtlo", bufs=3,
                                  name=f"xtl{s}")
                nc.sync.dma_start(out=lo[:], in_=xr_[:, 0:4, t0:t0 + NW])
                hi = xt_pool.tile([128, 4, NW], bf16, tag="xthi", bufs=3,
                                  name=f"xth{s}")
                nc.sync.dma_start(out=hi[:], in_=xr_[:, 4:8, t0:t0 + NW])
                return (lo, hi)

            def xt_k(xt, k):
                return xt[k // 4][:, k % 4, :]

            prev_c = [None] * 4
            for blk in range(NS // 2):
                sa, sb = 2 * blk, 2 * blk + 1
                xta = load_xt(sa)
                xtb = load_xt(sb)
                # paired GEMM: each (group, k) stationary feeds both slabs
                pse = [ps_pool.tile([128, NW], f32, tag="mm", name=f"pse{s}")
                       for s in (sa, sb)]
                for k in range(KT):
                    nc.tensor.matmul(pse[0][0:24, :], wck[k][:, 0:24], xt_k(xta, k),
                                     start=(k == 0), stop=(k == KT - 1))
                    nc.tensor.matmul(pse[1][0:24, :], wck[k][:, 0:24], xt_k(xtb, k),
                                     start=(k == 0), stop=(k == KT - 1))
                ext = [ext_pool.tile([24, NW], bf16, tag="ext", bufs=3,
                                     name=f"ext{s}") for s in (sa, sb)]
                nc.scalar.activation(ext[0][:], pse[0][0:24, :], AF.Copy)
                nc.scalar.activation(ext[1][:], pse[1][0:24, :], AF.Copy)
                ksb = [ksb_pool.tile([128, 4, NW], f32, tag="ksb", bufs=3,
                                     name=f"ksb{s}") for s in (sa, sb)]
                for mi in range(4):
                    c0 = 24 + mi * 128
                    ps = [ps_pool.tile([128, NW], f32, tag="mm",
                                       name=f"ps{s}_{mi}") for s in (sa, sb)]
                    for k in range(KT):
                        nc.tensor.matmul(ps[0][:], wck[k][:, c0:c0 + 128],
                                         xt_k(xta, k),
                                         start=(k == 0), stop=(k == KT - 1))
                        nc.tensor.matmul(ps[1][:], wck[k][:, c0:c0 + 128],
                                         xt_k(xtb, k),
                                         start=(k == 0), stop=(k == KT - 1))
                    nc.scalar.activation(ksb[0][:, mi, :], ps[0][:], AF.Copy)
                    nc.scalar.activation(ksb[1][:, mi, :], ps[1][:], AF.Copy)

                for si, s in enumerate((sa, sb)):
                    t0 = s * NW
                    ob = o_pool.tile([128, 4, NW], bf16, tag="ob", name=f"ob{s}")
                    for mi in range(4):
                        pb = psel_pool.tile([128, NW], f32, tag="pb",
                                            name=f"pb{s}_{mi}")
                        nc.tensor.matmul(pb[:], sel[:, 3 * mi + 0, :], ext[si][:],
                                         start=True, stop=True)
                        bexp = bx_pool.tile([128, NW], f32, tag="bexp",
                                            name=f"bx{s}_{mi}")
                        nc.scalar.activation(bexp[:], pb[:], AF.Sigmoid,
                                             bias=bb[:, mi:mi + 1])
                        pv = psel_pool.tile([128, NW], f32, tag="pv",
                                            name=f"pv{s}_{mi}")
                        nc.tensor.matmul(pv[:], sel[:, 3 * mi + 2, :], ext[si][:],
                                         start=True, stop=True)
                        pq = ps_pool.tile([128, NW], f32, tag="mm",
                                          name=f"pq{s}_{mi}")
                        nc.tensor.matmul(pq[:], sel[:, 3 * mi + 1, :], ext[si][:],
                                         start=True, stop=True)
                        qsb = qx_pool.tile([128, NW], bf16, tag="qsb",
                                           name=f"qx{s}_{mi}")
                        nc.scalar.activation(qsb[:], pq[:], AF.Copy)
                        u = u_pool.tile([128, NW], f32, tag="u", name=f"u{s}_{mi}")
                        nc.vector.tensor_mul(u[:], ksb[si][:, mi, :], pv[:])
                        c = c_pool.tile([128, NW], f32, tag=f"c{mi}", bufs=2,
                                        name=f"c{s}_{mi}")
                        init = 0.0 if s == 0 else prev_c[mi][:, NW - 1:NW]
                        nc.vector.tensor_tensor_scan(c[:], bexp[:], u[:], init,
                                                     ALU.mult, ALU.add)
                        prev_c[mi] = c
                        nc.vector.tensor_mul(ob[:, mi, :], c[:], qsb[:])
                        if mi == 1:
                            nc.sync.dma_start(
                                out=o_out.rearrange("(m p) s -> p m s", p=128)[:, 0:2, t0:t0 + NW],
                                in_=ob[:, 0:2, :])
                    nc.sync.dma_start(
                        out=o_out.rearrange("(m p) s -> p m s", p=128)[:, 2:4, t0:t0 + NW],
                        in_=ob[:, 2:4, :])
    nc.compile()
    return nc


def _build_l2(use_gb=True):
    import concourse.mybir as mybir
    from concourse import tile, bacc

    f32, bf16 = mybir.dt.float32, mybir.dt.bfloat16
    AF = mybir.ActivationFunctionType
    ALU = mybir.AluOpType

    nc = bacc.Bacc("TRN2", target_bir_lowering=False, debug=False, num_devices=8)
    oT = nc.dram_tensor("oT", [HID, TH], bf16, kind="ExternalInput")
    Wo = nc.dram_tensor("Wo", [HID, HID], bf16, kind="ExternalInput")
    xres = nc.dram_tensor("xres", [TH, HID], bf16, kind="ExternalInput")
    lng = nc.dram_tensor("lng", [1, HID], f32, kind="ExternalInput")
    lnb = nc.dram_tensor("lnb", [1, HID], f32, kind="ExternalInput")
    yout = nc.dram_tensor("yout", [TH, HID], bf16, kind="ExternalOutput")

    KT = 8
    MT = TH // 128  # 16 token tiles
    NW = 512

    with tile.TileContext(nc) as tc:
        with tc.tile_pool(name="wo", bufs=1) as wo_pool, \
             tc.tile_pool(name="ot", bufs=3) as ot_pool, \
             tc.tile_pool(name="xr", bufs=3) as xr_pool, \
             tc.tile_pool(name="y", bufs=2) as y_pool, \
             tc.tile_pool(name="dmp", bufs=2) as dmp_pool, \
             tc.tile_pool(name="z", bufs=3) as z_pool, \
             tc.tile_pool(name="st", bufs=4) as st_pool, \
             tc.tile_pool(name="ps", bufs=4, space="PSUM") as ps_pool:

            wok = []
            for k in range(KT):
                w1 = wo_pool.tile([128, HID], bf16, tag=f"wo{k}", name=f"wo{k}")
                nc.scalar.dma_start(out=w1[:], in_=Wo[k * 128:(k + 1) * 128, :])
                wok.append(w1)
            if use_gb:
                import concourse.bass as bass
                g_rep = wo_pool.tile([128, HID], f32)
                nc.gpsimd.dma_start(out=g_rep[:], in_=bass.AP(lng, 0, [[0, 128], [1, HID]]))
                b_rep = wo_pool.tile([128, HID], f32)
                nc.gpsimd.dma_start(out=b_rep[:], in_=bass.AP(lnb, 0, [[0, 128], [1, HID]]))

            for m in range(MT):
                ot = ot_pool.tile([128, KT, 128], bf16, tag="ot", name=f"ot{m}")
                nc.sync.dma_start(
                    out=ot[:],
                    in_=oT.rearrange("(kt p) s -> p kt s", p=128)[:, :, m * 128:(m + 1) * 128])
                xr = xr_pool.tile([128, HID], bf16, tag="xr", name=f"xr{m}")
                nc.sync.dma_start(out=xr[:], in_=xres[m * 128:(m + 1) * 128, :])

                y = y_pool.tile([128, HID], f32, tag="y", name=f"y{m}")
                stats = st_pool.tile([128, 8], f32, tag="stats", name=f"st{m}")
                ps0 = ps_pool.tile([128, NW], f32, tag="ps0", name=f"ps{m}_0")
                ps1 = ps_pool.tile([128, NW], f32, tag="ps1", name=f"ps{m}_1")
                # k outer, n inner: one LDWEIGHTS feeds both n-halves
                for k in range(KT):
                    nc.tensor.matmul(ps0[:], ot[:, k, :], wok[k][:, 0:NW],
                                     start=(k == 0), stop=(k == KT - 1))
                    nc.tensor.matmul(ps1[:], ot[:, k, :], wok[k][:, NW:HID],
                                     start=(k == 0), stop=(k == KT - 1))
                # y = psum + residual
                nc.vector.tensor_add(y[:, 0:NW], ps0[:], xr[:, 0:NW])
                nc.vector.tensor_add(y[:, NW:HID], ps1[:], xr[:, NW:HID])
                dump = dmp_pool.tile([128, HID], f32, tag="dump", name=f"dmp{m}")
                nc.scalar.activation(dump[:], y[:], AF.Copy, accum_out=stats[:, 1:2])
                dump2 = dmp_pool.tile([128, HID], f32, tag="dump2", name=f"dm2{m}")
                nc.scalar.activation(dump2[:], y[:], AF.Square, accum_out=stats[:, 2:3])
                # mu = s1/H ; var = s2/H - mu^2 ; rstd = 1/sqrt(var+eps)
                nc.vector.tensor_scalar_mul(stats[:, 3:4], stats[:, 1:2], 1.0 / HID)
                nc.vector.tensor_scalar_mul(stats[:, 4:5], stats[:, 2:3], 1.0 / HID)
                nc.vector.tensor_mul(stats[:, 5:6], stats[:, 3:4], stats[:, 3:4])
                nc.vector.tensor_scalar(stats[:, 6:7], stats[:, 4:5], stats[:, 5:6],
                                        EPS, ALU.subtract, ALU.add)
                nc.scalar.activation(stats[:, 6:7], stats[:, 6:7], AF.Sqrt)
                nc.vector.reciprocal(stats[:, 7:8], stats[:, 6:7])
                # z = (y - mu) * rstd ; out = z * g + b (g/b skipped when identity)
                if use_gb:
                    z = y_pool.tile([128, HID], f32, tag="z", name=f"z{m}")
                    nc.vector.tensor_scalar(z[:], y[:], stats[:, 3:4], stats[:, 7:8],
                                            ALU.subtract, ALU.mult)
                    zg = y_pool.tile([128, HID], f32, tag="zg", name=f"zg{m}")
                    nc.vector.tensor_mul(zg[:], z[:], g_rep[:])
                    out_t = z_pool.tile([128, HID], bf16, tag="out", name=f"o{m}")
                    nc.vector.tensor_add(out_t[:], zg[:], b_rep[:])
                else:
                    out_t = z_pool.tile([128, HID], bf16, tag="out", name=f"o{m}")
                    nc.vector.tensor_scalar(out_t[:], y[:], stats[:, 3:4],
                                            stats[:, 7:8], ALU.subtract, ALU.mult)
                nc.sync.dma_start(out=yout[m * 128:(m + 1) * 128, :], in_=out_t[:])

    nc.compile()
    return nc


_CACHE = {}


def _get_l1():
    if "l1" not in _CACHE:
        _CACHE["l1"] = _build_l1()
    return _CACHE["l1"]


def _get_l2(use_gb):
    key = ("l2", use_gb)
    if key not in _CACHE:
        _CACHE[key] = _build_l2(use_gb)
    return _CACHE[key]


LAST_EXEC_NS = None


def kernel(x, Wq, Wk, Wv, Wbeta, b_beta, Wo, b_o, ln_g, ln_b):
    import os
    from concourse.bass_utils import run_bass_kernel_spmd

    x = np.asarray(x, np.float32)
    Wq = np.asarray(Wq, np.float32); Wk = np.asarray(Wk, np.float32)
    Wv = np.asarray(Wv, np.float32); Wbeta = np.asarray(Wbeta, np.float32)
    b_beta = np.asarray(b_beta, np.float32); Wo = np.asarray(Wo, np.float32)
    b_o = np.asarray(b_o, np.float32)
    ln_g = np.asarray(ln_g, np.float32); ln_b = np.asarray(ln_b, np.float32)

    nc1 = _get_l1()
    use_gb = not (np.all(ln_g == 1.0) and np.all(ln_b == 0.0))
    nc2 = _get_l2(use_gb)
    trace = bool(os.environ.get("DELTANET_TRACE"))

    # column sums of Wq / Wv per head
    Wqs = Wq.reshape(HID, NH, D).sum(-1)   # (HID, NH)
    Wvs = Wv.reshape(HID, NH, D).sum(-1)

    xT = [np.ascontiguousarray(x[b].T).astype(BF) for b in range(B)]

    # selector blocks, shared by all cores
    selc = np.zeros((24, 12 * 128), np.float32)
    for mi in range(4):
        for f, row0 in enumerate((0, 8, 16)):  # beta-logit, qs, vs
            col0 = (3 * mi + f) * 128
            for p in range(128):
                selc[row0 + 2 * mi + p // 64, col0 + p] = 1.0
    selc = selc.astype(BF)

    in1 = []
    for c in range(8):
        b, hg = c // 2, c % 2
        hs = slice(hg * HG, (hg + 1) * HG)
        Wcat = np.concatenate(
            [Wbeta[:, hs], Wqs[:, hs], Wvs[:, hs], Wk[:, hg * HG * D:(hg + 1) * HG * D]],
            axis=1).astype(BF)
        bbh = b_beta[hs]
        bbx = np.empty((128, 4), np.float32)
        for mi in range(4):
            for p in range(128):
                bbx[p, mi] = bbh[2 * mi + p // 64]
        in1.append({
            "xT": xT[b],
            "Wcat": np.ascontiguousarray(Wcat),
            "bbx": bbx,
            "selc": selc,
        })
    if trace:
        import shutil
        for dpath in ("/root/problem/work/trace_l1", "/root/problem/work/trace_l2"):
            shutil.rmtree(dpath, ignore_errors=True)
            os.makedirs(dpath, exist_ok=True)
    kw1 = dict(trace=True, tmpdir="/root/problem/work/trace_l1") if trace else dict(trace=False)
    r1 = run_bass_kernel_spmd(nc1, in1, list(range(8)), **kw1)

    # assemble oT per batch: rows = hid (head-major), cols = tokens
    oT = [np.concatenate([np.asarray(r1.results[2 * b]["o_out"]),
                          np.asarray(r1.results[2 * b + 1]["o_out"])],
                         axis=0) for b in range(B)]

    Wo_b = Wo.astype(BF)
    in2 = []
    for c in range(8):
        b, half = c // 2, c % 2
        ts = slice(half * TH, (half + 1) * TH)
        in2.append({
            "oT": np.ascontiguousarray(oT[b][:, ts]),
            "Wo": Wo_b,
            "xres": np.ascontiguousarray(x[b, ts, :] + b_o).astype(BF),
            "lng": ln_g.reshape(1, HID),
            "lnb": ln_b.reshape(1, HID),
        })
    kw2 = dict(trace=True, tmpdir="/root/problem/work/trace_l2") if trace else dict(trace=False)
    r2 = run_bass_kernel_spmd(nc2, in2, list(range(8)), **kw2)

    global LAST_EXEC_NS
    LAST_EXEC_NS = (r1.exec_time_ns, r2.exec_time_ns)

    out = np.empty((B, S, HID), np.float32)
    for c in range(8):
        b, half = c // 2, c % 2
        out[b, half * TH:(half + 1) * TH, :] = np.asarray(
            r2.results[c]["yout"]).astype(np.float32)
    return out


# revision 18
# speedup vs baseline: 1.1166x; 1.1166x over previous
"""DeltaNet layer kernel for 8 Trainium2 NeuronCores.

Math note: in the reference's _delta_scan, the update added to the (D,D)
state h is identical for every row and h0=0, so all rows of h stay equal
forever. The layer therefore reduces exactly to a per-(batch, head)
first-order scalar-decay recurrence on a D-vector:

    c_t = beta_t * c_{t-1} + k_t * vsum_t,   o_t = qsum_t * c_t

with vsum = sum_d v, qsum = sum_d q. qsum/vsum only need x @ col-sums of
Wq/Wv. The recurrence maps 1:1 onto the DVE tensor_tensor_scan
instruction (fp32 state, one lane per (head, d) pair, scan along
tokens).

All GEMMs run in bf16 (fp32r measured ~2 cycles/row on HW vs bf16's 1;
bf16 also halves HBM traffic). Precision-sensitive pieces stay fp32:
the sigmoid/beta path (its error compounds through the recurrence, so
only pre-sigmoid LOGITS ever touch bf16), the scan state, and the k
tiles. One-shot values (qsum/vsum/o/residual/output) tolerate bf16.

Two SPMD launches on cores 0-7:
  L1: core (b, head-group of 8): GEMM1 -> extras (logit/qsum/vsum rows)
      + k rows; selector-matmul replication of the 3 extras fields to 64
      lanes/head (sigmoid fused into the beta evacuation, bias applied
      per-lane); u = k*vs; tensor_tensor_scan; o = qs*c -> bf16.
  L2: core (b, token-half): GEMM2 o @ Wo (b_o pre-folded into the
      residual host-side), fused residual-add + mean-accum on DVE,
      sumsq on ACT, LayerNorm, bf16 output upcast on host.
"""
import sys

sys.path.insert(0, "/opt/trn_rl_repo")

import numpy as np
import ml_dtypes

BF = ml_dtypes.bfloat16

B, S, HID, NH = 4, 4096, 1024, 16
D = HID // NH
EPS = 1e-5
HG = 8          # heads per L1 core
TH = S // 2     # tokens per L2 core


def _build_l1():
    import concourse.mybir as mybir
    from concourse import tile, bacc

    f32, bf16 = mybir.dt.float32, mybir.dt.bfloat16
    AF = mybir.ActivationFunctionType
    ALU = mybir.AluOpType

    nc = bacc.Bacc("TRN2", target_bir_lowering=False, debug=False, num_devices=8)
    xT = nc.dram_tensor("xT", [HID, S], bf16, kind="ExternalInput")
    # Wcat columns: [zb(8) | qs(8) | vs(8) | k(512)]
    Wcat = nc.dram_tensor("Wcat", [HID, 536], bf16, kind="ExternalInput")
    bbx = nc.dram_tensor("bbx", [128, 4], f32, kind="ExternalInput")
    # 12 selector blocks, order (mi, field): exp[p,:] row0+2mi+p//64 one-hot
    selc = nc.dram_tensor("selc", [24, 12 * 128], bf16, kind="ExternalInput")
    o_out = nc.dram_tensor("o_out", [HG * D, S], bf16, kind="ExternalOutput")

    KT = 8          # hid k-tiles
    NW = 512        # tokens per slab
    NS = S // NW

    with tile.TileContext(nc) as tc:
        with tc.tile_pool(name="wc", bufs=1) as wc_pool, \
             tc.tile_pool(name="xt", bufs=2) as xt_pool, \
             tc.tile_pool(name="ext", bufs=2) as ext_pool, \
             tc.tile_pool(name="ksb", bufs=2) as ksb_pool, \
             tc.tile_pool(name="bx", bufs=3) as bx_pool, \
             tc.tile_pool(name="qx", bufs=3) as qx_pool, \
             tc.tile_pool(name="u", bufs=3) as u_pool, \
             tc.tile_pool(name="c", bufs=2) as c_pool, \
             tc.tile_pool(name="osb", bufs=2) as o_pool, \
             tc.tile_pool(name="ps", bufs=4, space="PSUM") as ps_pool, \
             tc.tile_pool(name="psel", bufs=2, space="PSUM") as psel_pool:

            # weights on the ACT HWDGE queue so they overlap the first x
            # slab load on the sync queue; per-k tiles so the first matmul
            # only waits for chunk 0 (tile deps are whole-tile)
            wck = []
            for k in range(KT):
                w1 = wc_pool.tile([128, 536], bf16, tag=f"wc{k}", name=f"wc{k}")
                nc.scalar.dma_start(out=w1[:], in_=Wcat[k * 128:(k + 1) * 128, :])
                wck.append(w1)
            bb = wc_pool.tile([128, 4], f32)
            nc.gpsimd.dma_start(out=bb[:], in_=bbx[:])
            sel = wc_pool.tile([24, 12, 128], bf16)
            nc.gpsimd.dma_start(out=sel[:], in_=selc.rearrange("p (b l) -> p b l", l=128))

            xr_ = xT.rearrange("(kt p) s -> p kt s", p=128)

            def load_xt(s):
                t0 = s * NW
                lo = xt_pool.tile([128, 4, NW], bf16, tag="xtlo", bufs=3,
                                  name=f"xtl{s}")
                nc.sync.dma_start(out=lo[:], in_=xr_[:, 0:4, t0:t0 + NW])
                hi = xt_pool.tile([128, 4, NW], bf16, tag="xthi", bufs=3,
                                  name=f"xth{s}")
                nc.sync.dma_start(out=hi[:], in_=xr_[:, 4:8, t0:t0 + NW])
                return (lo, hi)

            def xt_k(xt, k):
                return xt[k // 4][:, k % 4, :]

            prev_c = [None] * 4
            for s in range(NS):
                t0 = s * NW
                xt = load_xt(s)
                pse = ps_pool.tile([128, NW], f32, tag="mm", name=f"pse{s}")
                for k in range(KT):
                    nc.tensor.matmul(pse[0:24, :], wck[k][:, 0:24], xt_k(xt, k),
                                     start=(k == 0), stop=(k == KT - 1))
                ext = ext_pool.tile([24, NW], bf16, tag="ext", name=f"ext{s}")
                nc.scalar.activation(ext[:], pse[0:24, :], AF.Copy)
                ksb = ksb_pool.tile([128, 4, NW], f32, tag="ksb", name=f"ksb{s}")
                for mi in range(4):
                    c0 = 24 + mi * 128
                    ps = ps_pool.tile([128, NW], f32, tag="mm", name=f"ps{s}_{mi}")
                    for k in range(KT):
                        nc.tensor.matmul(ps[:], wck[k][:, c0:c0 + 128],
                                         xt_k(xt, k),
                                         start=(k == 0), stop=(k == KT - 1))
                    nc.scalar.activation(ksb[:, mi, :], ps[:], AF.Copy)

                ob = o_pool.tile([128, 4, NW], bf16, tag="ob", name=f"ob{s}")
                for mi in range(4):
                    pb = psel_pool.tile([128, NW], f32, tag="pb", name=f"pb{s}_{mi}")
                    nc.tensor.matmul(pb[:], sel[:, 3 * mi + 0, :], ext[:],
                                     start=True, stop=True)
                    bexp = bx_pool.tile([128, NW], f32, tag="bexp", name=f"bx{s}_{mi}")
                    nc.scalar.activation(bexp[:], pb[:], AF.Sigmoid,
                                         bias=bb[:, mi:mi + 1])
                    pv = psel_pool.tile([128, NW], f32, tag="pv", name=f"pv{s}_{mi}")
                    nc.tensor.matmul(pv[:], sel[:, 3 * mi + 2, :], ext[:],
                                     start=True, stop=True)
                    pq = ps_pool.tile([128, NW], f32, tag="mm", name=f"pq{s}_{mi}")
                    nc.tensor.matmul(pq[:], sel[:, 3 * mi + 1, :], ext[:],
                                     start=True, stop=True)
                    qsb = qx_pool.tile([128, NW], bf16, tag="qsb", name=f"qx{s}_{mi}")
                    nc.scalar.activation(qsb[:], pq[:], AF.Copy)
                    u = u_pool.tile([128, NW], f32, tag="u", name=f"u{s}_{mi}")
                    nc.vector.tensor_mul(u[:], ksb[:, mi, :], pv[:])
                    c = c_pool.tile([128, NW], f32, tag=f"c{mi}", bufs=2,
                                    name=f"c{s}_{mi}")
                    init = 0.0 if s == 0 else prev_c[mi][:, NW - 1:NW]
                    nc.vector.tensor_tensor_scan(c[:], bexp[:], u[:], init,
                                                 ALU.mult, ALU.add)
                    prev_c[mi] = c
                    nc.vector.tensor_mul(ob[:, mi, :], c[:], qsb[:])
                    if mi == 1:
                        nc.sync.dma_start(
                            out=o_out.rearrange("(m p) s -> p m s", p=128)[:, 0:2, t0:t0 + NW],
                            in_=ob[:, 0:2, :])
                nc.sync.dma_start(
                    out=o_out.rearrange("(m p) s -> p m s", p=128)[:, 2:4, t0:t0 + NW],
                    in_=ob[:, 2:4, :])
    nc.compile()
    return nc


def _build_l2(use_gb=True):
    import concourse.mybir as mybir
    from concourse import tile, bacc

    f32, bf16 = mybir.dt.float32, mybir.dt.bfloat16
    AF = mybir.ActivationFunctionType
    ALU = mybir.AluOpType

    nc = bacc.Bacc("TRN2", target_bir_lowering=False, debug=False, num_devices=8)
    oT = nc.dram_tensor("oT", [HID, TH], bf16, kind="ExternalInput")
    Wo = nc.dram_tensor("Wo", [HID, HID], bf16, kind="ExternalInput")
    xres = nc.dram_tensor("xres", [TH, HID], bf16, kind="ExternalInput")
    lng = nc.dram_tensor("lng", [1, HID], f32, kind="ExternalInput")
    lnb = nc.dram_tensor("lnb", [1, HID], f32, kind="ExternalInput")
    yout = nc.dram_tensor("yout", [TH, HID], bf16, kind="ExternalOutput")

    KT = 8
    MT = TH // 128  # 16 token tiles
    NW = 512

    with tile.TileContext(nc) as tc:
        with tc.tile_pool(name="wo", bufs=1) as wo_pool, \
             tc.tile_pool(name="ot", bufs=3) as ot_pool, \
             tc.tile_pool(name="xr", bufs=3) as xr_pool, \
             tc.tile_pool(name="y", bufs=2) as y_pool, \
             tc.tile_pool(name="dmp", bufs=2) as dmp_pool, \
             tc.tile_pool(name="z", bufs=3) as z_pool, \
             tc.tile_pool(name="st", bufs=4) as st_pool, \
             tc.tile_pool(name="ps", bufs=4, space="PSUM") as ps_pool:

            wok = []
            for k in range(KT):
                w1 = wo_pool.tile([128, HID], bf16, tag=f"wo{k}", name=f"wo{k}")
                nc.scalar.dma_start(out=w1[:], in_=Wo[k * 128:(k + 1) * 128, :])
                wok.append(w1)
            if use_gb:
                import concourse.bass as bass
                g_rep = wo_pool.tile([128, HID], f32)
                nc.gpsimd.dma_start(out=g_rep[:], in_=bass.AP(lng, 0, [[0, 128], [1, HID]]))
                b_rep = wo_pool.tile([128, HID], f32)
                nc.gpsimd.dma_start(out=b_rep[:], in_=bass.AP(lnb, 0, [[0, 128], [1, HID]]))

            for m in range(MT):
                ot = ot_pool.tile([128, KT, 128], bf16, tag="ot", name=f"ot{m}")
                nc.sync.dma_start(
                    out=ot[:],
                    in_=oT.rearrange("(kt p) s -> p kt s", p=128)[:, :, m * 128:(m + 1) * 128])
                xr = xr_pool.tile([128, HID], bf16, tag="xr", name=f"xr{m}")
                nc.sync.dma_start(out=xr[:], in_=xres[m * 128:(m + 1) * 128, :])

                y = y_pool.tile([128, HID], f32, tag="y", name=f"y{m}")
                stats = st_pool.tile([128, 8], f32, tag="stats", name=f"st{m}")
                ps0 = ps_pool.tile([128, NW], f32, tag="ps0", name=f"ps{m}_0")
                ps1 = ps_pool.tile([128, NW], f32, tag="ps1", name=f"ps{m}_1")
                # k outer, n inner: one LDWEIGHTS feeds both n-halves
                for k in range(KT):
                    nc.tensor.matmul(ps0[:], ot[:, k, :], wok[k][:, 0:NW],
                                     start=(k == 0), stop=(k == KT - 1))
                    nc.tensor.matmul(ps1[:], ot[:, k, :], wok[k][:, NW:HID],
                                     start=(k == 0), stop=(k == KT - 1))
                # y = psum + residual
                nc.vector.tensor_add(y[:, 0:NW], ps0[:], xr[:, 0:NW])
                nc.vector.tensor_add(y[:, NW:HID], ps1[:], xr[:, NW:HID])
                dump = dmp_pool.tile([128, HID], f32, tag="dump", name=f"dmp{m}")
                nc.scalar.activation(dump[:], y[:], AF.Copy, accum_out=stats[:, 1:2])
                dump2 = dmp_pool.tile([128, HID], f32, tag="dump2", name=f"dm2{m}")
                nc.scalar.activation(dump2[:], y[:], AF.Square, accum_out=stats[:, 2:3])
                # mu = s1/H ; var = s2/H - mu^2 ; rstd = 1/sqrt(var+eps)
                nc.vector.tensor_scalar_mul(stats[:, 3:4], stats[:, 1:2], 1.0 / HID)
                nc.vector.tensor_scalar_mul(stats[:, 4:5], stats[:, 2:3], 1.0 / HID)
                nc.vector.tensor_mul(stats[:, 5:6], stats[:, 3:4], stats[:, 3:4])
                nc.vector.tensor_scalar(stats[:, 6:7], stats[:, 4:5], stats[:, 5:6],
                                        EPS, ALU.subtract, ALU.add)
                nc.scalar.activation(stats[:, 6:7], stats[:, 6:7], AF.Sqrt)
                nc.vector.reciprocal(stats[:, 7:8], stats[:, 6:7])
                # z = (y - mu) * rstd ; out = z * g + b (g/b skipped when identity)
                if use_gb:
                    z = y_pool.tile([128, HID], f32, tag="z", name=f"z{m}")
                    nc.vector.tensor_scalar(z[:], y[:], stats[:, 3:4], stats[:, 7:8],
                                            ALU.subtract, ALU.mult)
                    zg = y_pool.tile([128, HID], f32, tag="zg", name=f"zg{m}")
                    nc.vector.tensor_mul(zg[:], z[:], g_rep[:])
                    out_t = z_pool.tile([128, HID], bf16, tag="out", name=f"o{m}")
                    nc.vector.tensor_add(out_t[:], zg[:], b_rep[:])
                else:
                    out_t = z_pool.tile([128, HID], bf16, tag="out", name=f"o{m}")
                    nc.vector.tensor_scalar(out_t[:], y[:], stats[:, 3:4],
                                            stats[:, 7:8], ALU.subtract, ALU.mult)
                nc.sync.dma_start(out=yout[m * 128:(m + 1) * 128, :], in_=out_t[:])

    nc.compile()
    return nc


_CACHE = {}


def _get_l1():
    if "l1" not in _CACHE:
        _CACHE["l1"] = _build_l1()
    return _CACHE["l1"]


def _get_l2(use_gb):
    key = ("l2", use_gb)
    if key not in _CACHE:
        _CACHE[key] = _build_l2(use_gb)
    return _CACHE[key]


LAST_EXEC_NS = None


def kernel(x, Wq, Wk, Wv, Wbeta, b_beta, Wo, b_o, ln_g, ln_b):
    import os
    from concourse.bass_utils import run_bass_kernel_spmd

    x = np.asarray(x, np.float32)
    Wq = np.asarray(Wq, np.float32); Wk = np.asarray(Wk, np.float32)
    Wv = np.asarray(Wv, np.float32); Wbeta = np.asarray(Wbeta, np.float32)
    b_beta = np.asarray(b_beta, np.float32); Wo = np.asarray(Wo, np.float32)
    b_o = np.asarray(b_o, np.float32)
    ln_g = np.asarray(ln_g, np.float32); ln_b = np.asarray(ln_b, np.float32)

    nc1 = _get_l1()
    use_gb = not (np.all(ln_g == 1.0) and np.all(ln_b == 0.0))
    nc2 = _get_l2(use_gb)
    trace = bool(os.environ.get("DELTANET_TRACE"))

    # column sums of Wq / Wv per head
    Wqs = Wq.reshape(HID, NH, D).sum(-1)   # (HID, NH)
    Wvs = Wv.reshape(HID, NH, D).sum(-1)

    xT = [np.ascontiguousarray(x[b].T).astype(BF) for b in range(B)]

    # selector blocks, shared by all cores
    selc = np.zeros((24, 12 * 128), np.float32)
    for mi in range(4):
        for f, row0 in enumerate((0, 8, 16)):  # beta-logit, qs, vs
            col0 = (3 * mi + f) * 128
            for p in range(128):
                selc[row0 + 2 * mi + p // 64, col0 + p] = 1.0
    selc = selc.astype(BF)

    in1 = []
    for c in range(8):
        b, hg = c // 2, c % 2
        hs = slice(hg * HG, (hg + 1) * HG)
        Wcat = np.concatenate(
            [Wbeta[:, hs], Wqs[:, hs], Wvs[:, hs], Wk[:, hg * HG * D:(hg + 1) * HG * D]],
            axis=1).astype(BF)
        bbh = b_beta[hs]
        bbx = np.empty((128, 4), np.float32)
        for mi in range(4):
            for p in range(128):
                bbx[p, mi] = bbh[2 * mi + p // 64]
        in1.append({
            "xT": xT[b],
            "Wcat": np.ascontiguousarray(Wcat),
            "bbx": bbx,
            "selc": selc,
        })
    if trace:
        import shutil
        for dpath in ("/root/problem/work/trace_l1", "/root/problem/work/trace_l2"):
            shutil.rmtree(dpath, ignore_errors=True)
            os.makedirs(dpath, exist_ok=True)
    kw1 = dict(trace=True, tmpdir="/root/problem/work/trace_l1") if trace else dict(trace=False)
    r1 = run_bass_kernel_spmd(nc1, in1, list(range(8)), **kw1)

    # assemble oT per batch: rows = hid (head-major), cols = tokens
    oT = [np.concatenate([np.asarray(r1.results[2 * b]["o_out"]),
                          np.asarray(r1.results[2 * b + 1]["o_out"])],
                         axis=0) for b in range(B)]

    Wo_b = Wo.astype(BF)
    in2 = []
    for c in range(8):
        b, half = c // 2, c % 2
        ts = slice(half * TH, (half + 1) * TH)
        in2.append({
            "oT": np.ascontiguousarray(oT[b][:, ts]),
            "Wo": Wo_b,
            "xres": np.ascontiguousarray(x[b, ts, :] + b_o).astype(BF),
            "lng": ln_g.reshape(1, HID),
            "lnb": ln_b.reshape(1, HID),
        })
    kw2 = dict(trace=True, tmpdir="/root/problem/work/trace_l2") if trace else dict(trace=False)
    r2 = run_bass_kernel_spmd(nc2, in2, list(range(8)), **kw2)

    global LAST_EXEC_NS
    LAST_EXEC_NS = (r1.exec_time_ns, r2.exec_time_ns)

    out = np.empty((B, S, HID), np.float32)
    for c in range(8):
        b, half = c // 2, c % 2
        out[b, half * TH:(half + 1) * TH, :] = np.asarray(
            r2.results[c]["yout"]).astype(np.float32)
    return out


# revision 22
# speedup vs baseline: 1.1314x; 1.0132x over previous
"""DeltaNet layer kernel for 8 Trainium2 NeuronCores.

Math note: in the reference's _delta_scan, the update added to the (D,D)
state h is identical for every row and h0=0, so all rows of h stay equal
forever. The layer therefore reduces exactly to a per-(batch, head)
first-order scalar-decay recurrence on a D-vector:

    c_t = beta_t * c_{t-1} + k_t * vsum_t,   o_t = qsum_t * c_t

with vsum = sum_d v, qsum = sum_d q. qsum/vsum only need x @ col-sums of
Wq/Wv. The recurrence maps 1:1 onto the DVE tensor_tensor_scan
instruction (fp32 state, one lane per (head, d) pair, scan along
tokens).

All GEMMs run in bf16 (fp32r measured ~2 cycles/row on HW vs bf16's 1;
bf16 also halves HBM traffic). Precision-sensitive pieces stay fp32:
the sigmoid/beta path (its error compounds through the recurrence, so
only pre-sigmoid LOGITS ever touch bf16), the scan state, and the k
tiles. One-shot values (qsum/vsum/o/residual/output) tolerate bf16.

Two SPMD launches on cores 0-7:
  L1: core (b, head-group of 8): GEMM1 -> extras (logit/qsum/vsum rows)
      + k rows; selector-matmul replication of the 3 extras fields to 64
      lanes/head (sigmoid fused into the beta evacuation, bias applied
      per-lane); u = k*vs; tensor_tensor_scan; o = qs*c -> bf16.
  L2: core (b, token-half): GEMM2 o @ Wo (b_o pre-folded into the
      residual host-side), fused residual-add + mean-accum on DVE,
      sumsq on ACT, LayerNorm, bf16 output upcast on host.
"""
import sys

sys.path.insert(0, "/opt/trn_rl_repo")

import numpy as np
import ml_dtypes

BF = ml_dtypes.bfloat16




B, S, HID, NH = 4, 4096, 1024, 16
D = HID // NH
EPS = 1e-5
HG = 8          # heads per L1 core
TH = S // 2     # tokens per L2 core


def _build_l1():
    import concourse.mybir as mybir
    from concourse import tile, bacc

    f32, bf16 = mybir.dt.float32, mybir.dt.bfloat16
    AF = mybir.ActivationFunctionType
    ALU = mybir.AluOpType

    nc = bacc.Bacc("TRN2", target_bir_lowering=False, debug=False, num_devices=8)
    xT = nc.dram_tensor("xT", [HID, S], bf16, kind="ExternalInput")
    # Wcat columns: [zb(8) | qs(8) | vs(8) | k(512)]
    Wcat = nc.dram_tensor("Wcat", [HID, 536], bf16, kind="ExternalInput")
    bbx = nc.dram_tensor("bbx", [128, 4], f32, kind="ExternalInput")
    # 12 selector blocks, order (mi, field): exp[p,:] row0+2mi+p//64 one-hot
    selc = nc.dram_tensor("selc", [24, 12 * 128], bf16, kind="ExternalInput")
    o_out = nc.dram_tensor("o_out", [HG * D, S], bf16, kind="ExternalOutput")

    KT = 8          # hid k-tiles
    NW = 512        # tokens per slab
    NS = S // NW

    with tile.TileContext(nc) as tc:
        with tc.tile_pool(name="wc", bufs=1) as wc_pool, \
             tc.tile_pool(name="xt", bufs=2) as xt_pool, \
             tc.tile_pool(name="ext", bufs=2) as ext_pool, \
             tc.tile_pool(name="ksb", bufs=2) as ksb_pool, \
             tc.tile_pool(name="bx", bufs=3) as bx_pool, \
             tc.tile_pool(name="qx", bufs=3) as qx_pool, \
             tc.tile_pool(name="u", bufs=3) as u_pool, \
             tc.tile_pool(name="c", bufs=2) as c_pool, \
             tc.tile_pool(name="osb", bufs=2) as o_pool, \
             tc.tile_pool(name="ps", bufs=4, space="PSUM") as ps_pool, \
             tc.tile_pool(name="psel", bufs=2, space="PSUM") as psel_pool:

            # weights on the ACT HWDGE queue so they overlap the first x
            # slab load on the sync queue; per-k tiles so the first matmul
            # only waits for chunk 0 (tile deps are whole-tile)
            wck = []
            for k in range(KT):
                w1 = wc_pool.tile([128, 536], bf16, tag=f"wc{k}", name=f"wcb{k}")
                nc.scalar.dma_start(out=w1[:], in_=Wcat[k * 128:(k + 1) * 128, :])
                wck.append(w1)
            bb = wc_pool.tile([128, 4], f32)
            nc.gpsimd.dma_start(out=bb[:], in_=bbx[:])
            sel = wc_pool.tile([24, 12, 128], bf16)
            nc.gpsimd.dma_start(out=sel[:], in_=selc.rearrange("p (b l) -> p b l", l=128))

            xr_ = xT.rearrange("(kt p) s -> p kt s", p=128)

            def load_xt(s):
                t0 = s * NW
                lo = xt_pool.tile([128, 4, NW], bf16, tag="xtlo", bufs=3,
                                  name=f"xtl{s}")
                nc.sync.dma_start(out=lo[:], in_=xr_[:, 0:4, t0:t0 + NW])
                hi = xt_pool.tile([128, 4, NW], bf16, tag="xthi", bufs=3,
                                  name=f"xth{s}")
                nc.sync.dma_start(out=hi[:], in_=xr_[:, 4:8, t0:t0 + NW])
                return (lo, hi)

            def xt_k(xt, k):
                return xt[k // 4][:, k % 4, :]

            prev_c = [None] * 4
            for s in range(NS):
                t0 = s * NW
                xt = load_xt(s)
                pse = ps_pool.tile([128, NW], f32, tag="mm", name=f"pse{s}")
                for k in range(KT):
                    nc.tensor.matmul(pse[0:24, :], wck[k][:, 0:24], xt_k(xt, k),
                                     start=(k == 0), stop=(k == KT - 1))
                ext = ext_pool.tile([24, NW], bf16, tag="ext", name=f"ext{s}")
                nc.scalar.activation(ext[:], pse[0:24, :], AF.Copy)
                ksb = ksb_pool.tile([128, 4, NW], f32, tag="ksb", name=f"ksb{s}")
                for mi in range(4):
                    c0 = 24 + mi * 128
                    ps = ps_pool.tile([128, NW], f32, tag="mm", name=f"ps{s}_{mi}")
                    for k in range(KT):
                        nc.tensor.matmul(ps[:], wck[k][:, c0:c0 + 128],
                                         xt_k(xt, k),
                                         start=(k == 0), stop=(k == KT - 1))
                    nc.scalar.activation(ksb[:, mi, :], ps[:], AF.Copy)

                ob = o_pool.tile([128, 4, NW], bf16, tag="ob", name=f"ob{s}")
                for mi in range(4):
                    pb = psel_pool.tile([128, NW], f32, tag="pb", name=f"pb{s}_{mi}")
                    nc.tensor.matmul(pb[:], sel[:, 3 * mi + 0, :], ext[:],
                                     start=True, stop=True)
                    bexp = bx_pool.tile([128, NW], f32, tag="bexp", name=f"bx{s}_{mi}")
                    nc.scalar.activation(bexp[:], pb[:], AF.Sigmoid,
                                         bias=bb[:, mi:mi + 1])
                    pv = psel_pool.tile([128, NW], f32, tag="pv", name=f"pv{s}_{mi}")
                    nc.tensor.matmul(pv[:], sel[:, 3 * mi + 2, :], ext[:],
                                     start=True, stop=True)
                    pq = ps_pool.tile([128, NW], f32, tag="mm", name=f"pq{s}_{mi}")
                    nc.tensor.matmul(pq[:], sel[:, 3 * mi + 1, :], ext[:],
                                     start=True, stop=True)
                    qsb = qx_pool.tile([128, NW], bf16, tag="qsb", name=f"qx{s}_{mi}")
                    nc.scalar.activation(qsb[:], pq[:], AF.Copy)
                    u = u_pool.tile([128, NW], f32, tag="u", name=f"u{s}_{mi}")
                    nc.vector.tensor_mul(u[:], ksb[:, mi, :], pv[:])
                    c = c_pool.tile([128, NW], f32, tag=f"c{mi}", bufs=2,
                                    name=f"c{s}_{mi}")
                    init = 0.0 if s == 0 else prev_c[mi][:, NW - 1:NW]
                    nc.vector.tensor_tensor_scan(c[:], bexp[:], u[:], init,
                                                 ALU.mult, ALU.add)
                    prev_c[mi] = c
                    nc.vector.tensor_mul(ob[:, mi, :], c[:], qsb[:])
                    if mi == 1:
                        nc.sync.dma_start(
                            out=o_out.rearrange("(m p) s -> p m s", p=128)[:, 0:2, t0:t0 + NW],
                            in_=ob[:, 0:2, :])
                nc.sync.dma_start(
                    out=o_out.rearrange("(m p) s -> p m s", p=128)[:, 2:4, t0:t0 + NW],
                    in_=ob[:, 2:4, :])
    nc.compile()
    return nc


def _build_l2(use_gb=True):
    import concourse.mybir as mybir
    from concourse import tile, bacc

    f32, bf16 = mybir.dt.float32, mybir.dt.bfloat16
    AF = mybir.ActivationFunctionType
    ALU = mybir.AluOpType

    nc = bacc.Bacc("TRN2", target_bir_lowering=False, debug=False, num_devices=8)
    oT = nc.dram_tensor("oT", [HID, TH], bf16, kind="ExternalInput")
    Wo = nc.dram_tensor("Wo", [HID, HID], bf16, kind="ExternalInput")
    xres = nc.dram_tensor("xres", [TH, HID], bf16, kind="ExternalInput")
    lng = nc.dram_tensor("lng", [1, HID], f32, kind="ExternalInput")
    lnb = nc.dram_tensor("lnb", [1, HID], f32, kind="ExternalInput")
    yout = nc.dram_tensor("yout", [TH, HID], bf16, kind="ExternalOutput")

    KT = 8
    MT = TH // 128  # 16 token tiles
    NW = 512

    with tile.TileContext(nc) as tc:
        with tc.tile_pool(name="wo", bufs=1) as wo_pool, \
             tc.tile_pool(name="ot", bufs=3) as ot_pool, \
             tc.tile_pool(name="xr", bufs=3) as xr_pool, \
             tc.tile_pool(name="y", bufs=2) as y_pool, \
             tc.tile_pool(name="dmp", bufs=2) as dmp_pool, \
             tc.tile_pool(name="z", bufs=3) as z_pool, \
             tc.tile_pool(name="st", bufs=4) as st_pool, \
             tc.tile_pool(name="ps", bufs=4, space="PSUM") as ps_pool:

            wok = []
            for k in range(KT):
                w1 = wo_pool.tile([128, HID], bf16, tag=f"wo{k}", name=f"wo{k}")
                nc.scalar.dma_start(out=w1[:], in_=Wo[k * 128:(k + 1) * 128, :])
                wok.append(w1)
            if use_gb:
                import concourse.bass as bass
                g_rep = wo_pool.tile([128, HID], f32)
                nc.gpsimd.dma_start(out=g_rep[:], in_=bass.AP(lng, 0, [[0, 128], [1, HID]]))
                b_rep = wo_pool.tile([128, HID], f32)
                nc.gpsimd.dma_start(out=b_rep[:], in_=bass.AP(lnb, 0, [[0, 128], [1, HID]]))

            for m in range(MT):
                ot = ot_pool.tile([128, KT, 128], bf16, tag="ot", bufs=4,
                                  name=f"ot{m}")
                nc.sync.dma_start(
                    out=ot[:],
                    in_=oT.rearrange("(kt p) s -> p kt s", p=128)[:, :, m * 128:(m + 1) * 128])
                xr = xr_pool.tile([128, HID], bf16, tag="xr", bufs=4, name=f"xr{m}")
                nc.scalar.dma_start(out=xr[:], in_=xres[m * 128:(m + 1) * 128, :])

                y = y_pool.tile([128, HID], f32, tag="y", name=f"y{m}")
                stats = st_pool.tile([128, 8], f32, tag="stats", name=f"st{m}")
                ps0 = ps_pool.tile([128, NW], f32, tag="ps0", name=f"ps{m}_0")
                ps1 = ps_pool.tile([128, NW], f32, tag="ps1", name=f"ps{m}_1")
                # k outer, n inner: one LDWEIGHTS feeds both n-halves
                for k in range(KT):
                    nc.tensor.matmul(ps0[:], ot[:, k, :], wok[k][:, 0:NW],
                                     start=(k == 0), stop=(k == KT - 1))
                    nc.tensor.matmul(ps1[:], ot[:, k, :], wok[k][:, NW:HID],
                                     start=(k == 0), stop=(k == KT - 1))
                # y = psum + residual
                nc.vector.tensor_add(y[:, 0:NW], ps0[:], xr[:, 0:NW])
                nc.vector.tensor_add(y[:, NW:HID], ps1[:], xr[:, NW:HID])
                dump = dmp_pool.tile([128, HID], f32, tag="dump", name=f"dmp{m}")
                nc.scalar.activation(dump[:], y[:], AF.Copy, accum_out=stats[:, 1:2])
                dump2 = dmp_pool.tile([128, HID], f32, tag="dump2", name=f"dm2{m}")
                nc.scalar.activation(dump2[:], y[:], AF.Square, accum_out=stats[:, 2:3])
                # mu = s1/H ; var = s2/H - mu^2 ; rstd = 1/sqrt(var+eps)
                nc.vector.tensor_scalar_mul(stats[:, 3:4], stats[:, 1:2], 1.0 / HID)
                nc.vector.tensor_scalar_mul(stats[:, 4:5], stats[:, 2:3], 1.0 / HID)
                nc.vector.tensor_mul(stats[:, 5:6], stats[:, 3:4], stats[:, 3:4])
                nc.vector.tensor_scalar(stats[:, 6:7], stats[:, 4:5], stats[:, 5:6],
                                        EPS, ALU.subtract, ALU.add)
                nc.scalar.activation(stats[:, 6:7], stats[:, 6:7], AF.Sqrt)
                nc.vector.reciprocal(stats[:, 7:8], stats[:, 6:7])
                # z = (y - mu) * rstd ; out = z * g + b (g/b skipped when identity)
                if use_gb:
                    z = y_pool.tile([128, HID], f32, tag="z", name=f"z{m}")
                    nc.vector.tensor_scalar(z[:], y[:], stats[:, 3:4], stats[:, 7:8],
                                            ALU.subtract, ALU.mult)
                    zg = y_pool.tile([128, HID], f32, tag="zg", name=f"zg{m}")
                    nc.vector.tensor_mul(zg[:], z[:], g_rep[:])
                    out_t = z_pool.tile([128, HID], bf16, tag="out", name=f"o{m}")
                    nc.vector.tensor_add(out_t[:], zg[:], b_rep[:])
                else:
                    out_t = z_pool.tile([128, HID], bf16, tag="out", name=f"o{m}")
                    nc.vector.tensor_scalar(out_t[:], y[:], stats[:, 3:4],
                                            stats[:, 7:8], ALU.subtract, ALU.mult)
                nc.sync.dma_start(out=yout[m * 128:(m + 1) * 128, :], in_=out_t[:])

    nc.compile()
    return nc


_CACHE = {}


def _get_l1():
    if "l1" not in _CACHE:
        _CACHE["l1"] = _build_l1()
    return _CACHE["l1"]


def _get_l2(use_gb):
    key = ("l2", use_gb)
    if key not in _CACHE:
        _CACHE[key] = _build_l2(use_gb)
    return _CACHE[key]


LAST_EXEC_NS = None


def kernel(x, Wq, Wk, Wv, Wbeta, b_beta, Wo, b_o, ln_g, ln_b):
    import os
    from concourse.bass_utils import run_bass_kernel_spmd

    x = np.asarray(x, np.float32)
    Wq = np.asarray(Wq, np.float32); Wk = np.asarray(Wk, np.float32)
    Wv = np.asarray(Wv, np.float32); Wbeta = np.asarray(Wbeta, np.float32)
    b_beta = np.asarray(b_beta, np.float32); Wo = np.asarray(Wo, np.float32)
    b_o = np.asarray(b_o, np.float32)
    ln_g = np.asarray(ln_g, np.float32); ln_b = np.asarray(ln_b, np.float32)

    nc1 = _get_l1()
    use_gb = not (np.all(ln_g == 1.0) and np.all(ln_b == 0.0))
    nc2 = _get_l2(use_gb)
    trace = bool(os.environ.get("DELTANET_TRACE"))

    # column sums of Wq / Wv per head
    Wqs = Wq.reshape(HID, NH, D).sum(-1)   # (HID, NH)
    Wvs = Wv.reshape(HID, NH, D).sum(-1)

    xT = [np.ascontiguousarray(x[b].T).astype(BF) for b in range(B)]

    # selector blocks, shared by all cores
    selc = np.zeros((24, 12 * 128), np.float32)
    for mi in range(4):
        for f, row0 in enumerate((0, 8, 16)):  # beta-logit, qs, vs
            col0 = (3 * mi + f) * 128
            for p in range(128):
                selc[row0 + 2 * mi + p // 64, col0 + p] = 1.0
    selc = selc.astype(BF)

    in1 = []
    for c in range(8):
        b, hg = c // 2, c % 2
        hs = slice(hg * HG, (hg + 1) * HG)
        Wcat = np.concatenate(
            [Wbeta[:, hs], Wqs[:, hs], Wvs[:, hs], Wk[:, hg * HG * D:(hg + 1) * HG * D]],
            axis=1).astype(BF)
        bbh = b_beta[hs]
        bbx = np.empty((128, 4), np.float32)
        for mi in range(4):
            for p in range(128):
                bbx[p, mi] = bbh[2 * mi + p // 64]
        in1.append({
            "xT": xT[b],
            "Wcat": np.ascontiguousarray(Wcat),
            "bbx": bbx,
            "selc": selc,
        })
    if trace:
        import shutil
        for dpath in ("/root/problem/work/trace_l1", "/root/problem/work/trace_l2"):
            shutil.rmtree(dpath, ignore_errors=True)
            os.makedirs(dpath, exist_ok=True)
    kw1 = dict(trace=True, tmpdir="/root/problem/work/trace_l1") if trace else dict(trace=False)
    r1 = run_bass_kernel_spmd(nc1, in1, list(range(8)), **kw1)

    # assemble oT per batch: rows = hid (head-major), cols = tokens
    oT = [np.concatenate([np.asarray(r1.results[2 * b]["o_out"]),
                          np.asarray(r1.results[2 * b + 1]["o_out"])],
                         axis=0) for b in range(B)]

    Wo_b = Wo.astype(BF)
    in2 = []
    for c in range(8):
        b, half = c // 2, c % 2
        ts = slice(half * TH, (half + 1) * TH)
        in2.append({
            "oT": np.ascontiguousarray(oT[b][:, ts]),
            "Wo": Wo_b,
            "xres": np.ascontiguousarray(x[b, ts, :] + b_o).astype(BF),
            "lng": ln_g.reshape(1, HID),
            "lnb": ln_b.reshape(1, HID),
        })
    kw2 = dict(trace=True, tmpdir="/root/problem/work/trace_l2") if trace else dict(trace=False)
    r2 = run_bass_kernel_spmd(nc2, in2, list(range(8)), **kw2)

    global LAST_EXEC_NS
    LAST_EXEC_NS = (r1.exec_time_ns, r2.exec_time_ns)

    out = np.empty((B, S, HID), np.float32)
    for c in range(8):
        b, half = c // 2, c % 2
        out[b, half * TH:(half + 1) * TH, :] = np.asarray(
            r2.results[c]["yout"]).astype(np.float32)
    return out
